# revision 1
# baseline (speedup 1.0000x reference)
"""YIN pitch Trainium2 kernel, Phase 2: PE band-matmul difference function.

C[f,tau] = sum_n x[n]*x[n+tau]*[80f <= n <= 80f+132] on the tensor engine:
contraction over 128-sample tiles (k = partition = sample), stationary
operand = x-valued band selector slab [128, 32] (<=4 active frame columns,
zero padded; slab positions repeat with period 20 tiles), moving operand =
Hankel slice XD[:, 128t+1 : 128t+134] where XD[p, c] = x[p+c], streamed from
a DRAM bounce buffer in fp8.  PSUM accumulates 32-frame windows (out rows
always [0, 32) - PE requires 32-aligned PSUM base partitions).

Energy terms + CMNDF threshold pick stay on DVE in f32.
"""

import math

import numpy as np

import bass_rust
import concourse.bass as bass
import concourse.mybir as mybir
import concourse.tile as tile
from concourse.bass_utils import run_bass_kernel_spmd
from concourse.tile_rust import add_dep_helper

_WAIT_LIM = 1


def _split_excess_waits(nc):
    uid = 0
    for fn in nc.m.functions:
        for blk in fn.blocks:
            out = []
            changed = False
            for inst in blk.instructions:
                si = inst.sync_info
                waits = list(si.on_wait) if si is not None and si.on_wait else []
                if len(waits) > _WAIT_LIM:
                    changed = True
                    extra = waits[:-_WAIT_LIM]
                    si.on_wait = waits[-_WAIT_LIM:]
                    for j in range(0, len(extra), _WAIT_LIM):
                        nop = bass_rust.InstNoOp(name=f"WSPLIT-{uid}", ins=[], outs=[])
                        uid += 1
                        nop.engine = inst.engine
                        nop.sync_info = bass_rust.SyncInfo(
                            on_wait=extra[j:j + _WAIT_LIM], on_update=[]
                        )
                        out.append(nop)
                out.append(inst)
            if changed:
                blk.instructions = out


def _short_drain_and_barrier(self, tick_clock, wait_clock):
    # Tail with a single all-engine barrier: drain, barrier, sem cleanup.
    # The trailing barrier of the stock TileContext tail only re-syncs
    # engines that have no further work; the runtime joins engines anyway.
    from concourse.vector_clock import ScopedClock
    nc = self.nc
    drain_inst = nc.sync.drain()
    wait_clock.add_sem_waits(
        drain_inst.ins, ScopedClock({None: tick_clock.global_clock})
    )
    nc.all_engine_barrier()
    assert self.sems is not None
    popped = nc._tile_sem_poison_stack.pop()
    assert popped is self._sem_poison
    nc.clear_and_free_semaphores(list(self.sems.allocated().values()))


tile.TileContext._drain_and_barrier = _short_drain_and_barrier


B = 8
N = 80000
SR = 8000
HOP = 80
TAU_MIN = 20
TAU_MAX = 133
W = 133
FRAME_LEN = 266
N_FRAMES = 997
N_OUT = 996          # frames 0..995 are emitted
THRESH = 0.2
EPS = 1e-8
BIG = 1.0e9

N_BLK = 8
FT = 268
G = 4                # max frames per 128-sample tile
NT = 625             # sample tiles
NCHUNK = 640         # xpad chunk width: [128, 640] covers 81920 samples
SEG_T = 128          # tiles per XD segment
SEG_LEN = SEG_T * 128 + TAU_MAX    # 3333
N_SEG = 5
WIN = 32             # frames per PSUM window
PERIOD = 20          # slab-position periodicity in tiles

F32 = mybir.dt.float32
BF16 = mybir.dt.bfloat16
DT_LOW = mybir.dt.float8e4   # PE operand dtype (e4m3); set BF16 to fall back
AluOp = mybir.AluOpType
Axis = mybir.AxisListType


def _ap(t, offset, pairs):
    return bass.AP(t, offset, pairs)


def _sap(tile_ap, offset, pairs):
    """AP on an SBUF tile: partition pair step = row pitch (elements)."""
    pitch = tile_ap[:, 0:1].ap[0][0]
    return bass.AP(tile_ap.tensor, offset, [[pitch, pairs[0][1]]] + pairs[1:])


def _fb(t):
    return math.ceil((128 * t - (W - 1)) / HOP)


def _geometry():
    """Period-5 cover mask + period-20 slab groups."""
    mask5 = np.zeros((128, 5, G), np.float32)
    for r in range(5):
        n0 = 128 * r
        fb = _fb(r)
        for g in range(G):
            f = fb + g
            lo = max(0, HOP * f - n0)
            hi = min(127, HOP * f + (W - 1) - n0)
            if lo <= hi:
                mask5[lo:hi + 1, r, g] = 1.0

    t_eff = max(t for t in range(NT) if _fb(t) <= N_OUT - 1)
    groups = []
    for rho in range(PERIOD):
        fb = _fb(rho)
        byw = {}
        for g in range(G):
            byw.setdefault((fb + g) // WIN, []).append(g)
        for a_off, gs in sorted(byw.items()):
            groups.append(
                dict(rho=rho, a_off=a_off, glo=min(gs), ghi=max(gs),
                     pos=(fb + min(gs)) - WIN * a_off)
            )
    return mask5, groups, t_eff


def _build_nc():
    nc = bass.Bass(trn_type="TRN2")
    x_d = nc.dram_tensor("x", [N], F32, kind="ExternalInput")
    f0_d = nc.dram_tensor("f0", [N_OUT], F32, kind="ExternalOutput")

    mask5, groups, t_eff = _geometry()
    n_groups = len(groups)
    for gi, gr in enumerate(groups):
        gr["nv"] = (t_eff - gr["rho"]) // PERIOD + 1
        gr["gi"] = gi
    by_rho = {}
    for gr in groups:
        by_rho.setdefault(gr["rho"], []).append(gr)

    tau_row = np.arange(1, TAU_MAX + 1, dtype=np.float32)
    tauc_d = nc.inline_tensor(np.broadcast_to(tau_row, (128, W)).copy(), name="tauc")
    taubig_d = nc.inline_tensor(
        (np.broadcast_to(tau_row, (128, W)) + np.float32(BIG)).astype(np.float32),
        name="taubig",
    )
    ident_d = nc.inline_tensor(np.eye(128, dtype=np.float32), name="ident")
    mask_d = nc.inline_tensor(
        mask5.reshape(128, 5 * G).astype(np.dtype(mybir.dt.np(BF16))), name="bmask"
    )
    zl_d = nc.inline_tensor(
        np.zeros((1, WIN), dtype=np.dtype(mybir.dt.np(DT_LOW))), name="zl"
    )
    zr_d = nc.inline_tensor(
        np.zeros((1, W), dtype=np.dtype(mybir.dt.np(DT_LOW))), name="zr"
    )

    # pieces per tile -> windows per pair; win_last in pair units
    def _pieces(t):
        fb = _fb(t)
        byw = {}
        for g in range(G):
            byw.setdefault((fb + g) // WIN, []).append(g)
        return [
            dict(a=a, glo=min(gs), ghi=max(gs), pos=(fb + min(gs)) - WIN * a)
            for a, gs in sorted(byw.items())
        ]

    pair_wins = {}
    win_last = {}
    for t2 in range(313):
        wins = set()
        for t in (2 * t2, 2 * t2 + 1):
            if t > t_eff:
                continue
            for pc in _pieces(t):
                f_lo = max(_fb(t) + pc["glo"], 0)
                f_hi = min(_fb(t) + pc["ghi"], N_OUT - 1)
                if f_lo <= f_hi and pc["a"] >= 0:
                    wins.add(pc["a"])
        if wins:
            pair_wins[t2] = tuple(sorted(wins))
            for a in wins:
                win_last[a] = t2

    with tile.TileContext(nc) as tc:
        with (
            tc.tile_pool(name="persist", bufs=1) as pp,
            tc.tile_pool(name="work", bufs=2) as wp,
            tc.tile_pool(name="xdpool", bufs=3) as xdp,
            tc.tile_pool(name="psum", bufs=6, space="PSUM") as psp,
            tc.tile_pool(name="ps2", bufs=1, space="PSUM") as ps2,
            tc.tile_pool(name="dram", bufs=1, space="DRAM") as dp,
        ):
            # ---- weight-slab zero fill first: overlaps the entire x chain
            xb = pp.tile([128, 640 * 64], DT_LOW)
            nc.gpsimd.memset(xb[:].bitcast(F32), 0.0)

            # ---- constants to SBUF
            tauc = pp.tile([128, W], F32)
            nc.scalar.dma_start(tauc[:], tauc_d[:])
            taubig = pp.tile([128, W], F32)
            nc.scalar.dma_start(taubig[:], taubig_d[:])
            ident = pp.tile([128, 128], F32)
            nc.scalar.dma_start(ident[:], ident_d[:])
            bmask = pp.tile([128, 5 * G], BF16)
            nc.sync.dma_start(bmask[:], mask_d[:])
            zl = pp.tile([1, WIN], DT_LOW)
            nc.scalar.dma_start(zl[:], zl_d[:])
            zr = pp.tile([1, W], DT_LOW)
            nc.scalar.dma_start(zr[:], zr_d[:])
            f0all = pp.tile([128, N_BLK], F32)
            nc.vector.memset(f0all[:], 0.0)

            # ---- x -> chunked SBUF (f32), convert, bounce to DRAM.
            # Order matters: the transpose-DMA switches the DMA xbar mode and
            # serializes against every in-flight DMACopy, so it runs before
            # the large copies.
            xchunk = pp.tile([128, NCHUNK], F32)
            nc.vector.memset(xchunk[:], 0.0)
            nc.sync.dma_start(
                xchunk[0:125, 0:NCHUNK],
                _ap(x_d, 0, [[NCHUNK, 125], [1, NCHUNK]]),
            )
            xbf = pp.tile([128, NCHUNK], BF16)
            nc.vector.tensor_copy(xbf[:], xchunk[:])
            xpad16_d = dp.tile([128, NCHUNK], BF16)
            nc.sync.dma_start(xpad16_d[:], xbf[:])
            xpm16 = pp.tile([128, NCHUNK], BF16)
            _tr = nc.sync.dma_start(
                xpm16[:],
                _ap(xpad16_d.tensor, 0, [[128, NCHUNK], [1, 128]]),
                transpose=True,
            )
            xlow = pp.tile([128, NCHUNK], DT_LOW)
            nc.vector.tensor_copy(xlow[:], xchunk[:])
            xpad8_d = dp.tile([128, NCHUNK], DT_LOW)
            nc.sync.dma_start(xpad8_d[:], xlow[:])

            # ---- weight slabs, t-major with window-parity slots:
            # tile t, window a piece -> cols [64 t + 32 (a%2) + pos, +ncols)
            for gr in groups:
                rho, nv = gr["rho"], gr["nv"]
                ncols = gr["ghi"] - gr["glo"] + 1
                for phi in (0, 1):  # v parity (slot alternates with v)
                    nu = (nv - phi + 1) // 2
                    if nu <= 0:
                        continue
                    slot = (gr["a_off"] + phi) % 2
                    base = 64 * (PERIOD * phi + rho) + 32 * slot + gr["pos"]
                    nc.vector.tensor_tensor(
                        out=_sap(xb, base, [[1, 128], [128 * PERIOD, nu], [1, ncols]]),
                        in0=_sap(xpm16, PERIOD * phi + rho,
                                 [[1, 128], [2 * PERIOD, nu], [0, ncols]]),
                        in1=_sap(bmask, (rho % 5) * G + gr["glo"],
                                 [[1, 128], [0, nu], [1, ncols]]),
                        op=AluOp.mult,
                    )
                # clip frames < 0 or > N_OUT-1 (first/last slots only)
                for v in (0, nv - 1):
                    t = PERIOD * v + rho
                    if t > t_eff:
                        continue
                    slot = (gr["a_off"] + v) % 2
                    for g in range(gr["glo"], gr["ghi"] + 1):
                        f = _fb(t) + g
                        if 0 <= f <= N_OUT - 1:
                            continue
                        col = 64 * t + 32 * slot + gr["pos"] + (g - gr["glo"])
                        nc.vector.memset(
                            _sap(xb, col, [[1, 128], [1, 1]]), 0.0
                        )

            # ---- E-path tiles per block (f32)
            xfr = {}
            qq = {}
            for b in range(N_BLK):
                Rb = 128 if b < N_BLK - 1 else N_OUT - 128 * (N_BLK - 1)
                xfr[b] = wp.tile([128, FT], F32, tag=f"xfr{b}", name=f"xfr{b}")
                nc.scalar.dma_start(
                    xfr[b][:Rb, :],
                    _ap(x_d, HOP * 128 * b, [[HOP, Rb], [1, FT]]),
                )
                sq = wp.tile([128, FRAME_LEN], F32, tag="sq")
                nc.scalar.square(sq[:Rb, :], xfr[b][:Rb, :FRAME_LEN])
                qq[b] = wp.tile([128, FRAME_LEN], F32, tag=f"qq{b}", name=f"qq{b}")
                nc.vector.tensor_tensor_scan(
                    qq[b][:Rb, :], sq[:Rb, :], sq[:Rb, :], 0.0,
                    AluOp.add, AluOp.bypass,
                )

            csb = {}
            for b in range(N_BLK):
                csb[b] = wp.tile([128, W], F32, tag=f"csb{b}", name=f"csb{b}")
            blk_done = {b: 0 for b in range(N_BLK)}

            def finish_block(b):
                Rb = 128 if b < N_BLK - 1 else N_OUT - 128 * (N_BLK - 1)
                e2 = wp.tile([128, W], F32, tag="e2")
                nc.vector.tensor_sub(
                    e2[:Rb, :], qq[b][:Rb, W:FRAME_LEN], qq[b][:Rb, 0:W]
                )
                d = wp.tile([128, W], F32, tag="d")
                nc.vector.scalar_tensor_tensor(
                    out=d[:Rb, :], in0=csb[b][:Rb, :], scalar=-2.0, in1=e2[:Rb, :],
                    op0=AluOp.mult, op1=AluOp.add,
                )
                nc.vector.tensor_scalar_add(d[:Rb, :], d[:Rb, :], qq[b][:Rb, W - 1:W])
                cum = wp.tile([128, W], F32, tag="cum")
                nc.vector.tensor_tensor_scan(
                    cum[:Rb, :], d[:Rb, :], d[:Rb, :], 0.0, AluOp.add, AluOp.bypass
                )
                lhs = wp.tile([128, W], F32, tag="lhs")
                nc.vector.tensor_mul(lhs[:Rb, :], d[:Rb, :], tauc[:Rb, :])
                rhs = wp.tile([128, W], F32, tag="rhs")
                nc.vector.tensor_scalar(
                    out=rhs[:Rb, :], in0=cum[:Rb, :], scalar1=EPS, scalar2=THRESH,
                    op0=AluOp.max, op1=AluOp.mult,
                )
                cand = wp.tile([128, W], F32, tag="cand")
                nc.vector.tensor_tensor(
                    out=cand[:Rb, :], in0=lhs[:Rb, :], in1=rhs[:Rb, :], op=AluOp.is_lt
                )
                v = wp.tile([128, W], F32, tag="v")
                nc.vector.scalar_tensor_tensor(
                    out=v[:Rb, :], in0=cand[:Rb, :], scalar=-BIG, in1=taubig[:Rb, :],
                    op0=AluOp.mult, op1=AluOp.add,
                )
                tmin = wp.tile([128, 1], F32, tag="tmin")
                nc.vector.tensor_reduce(
                    tmin[:Rb, :], v[:Rb, TAU_MIN - 1:W], axis=Axis.X, op=AluOp.min
                )
                voi = wp.tile([128, 1], F32, tag="voi")
                nc.vector.tensor_scalar(
                    out=voi[:Rb, :], in0=tmin[:Rb, :], scalar1=BIG * 0.5,
                    scalar2=None, op0=AluOp.is_lt,
                )
                rec = wp.tile([128, 1], F32, tag="rec")
                nc.vector.reciprocal(rec[:Rb, :], tmin[:Rb, :])
                f0v = wp.tile([128, 1], F32, tag="f0v")
                nc.vector.tensor_scalar(
                    out=f0v[:Rb, :], in0=rec[:Rb, :], scalar1=float(SR),
                    scalar2=None, op0=AluOp.mult,
                )
                nc.vector.tensor_mul(f0all[:Rb, b:b + 1], f0v[:Rb, :], voi[:Rb, :])

            # ---- band matmuls over XD segments (fp8 DoubleRow pairs)
            cps = {}
            DR = mybir.MatmulPerfMode.DoubleRow
            # progressive segments: small first chunk so matmuls start early
            seg_bounds = [0, 64, 192, 320, 448, 544, 608, 640]
            for si in range(len(seg_bounds) - 1):
                t0 = seg_bounds[si]
                if t0 > t_eff:
                    break
                seg_t = seg_bounds[si + 1] - t0
                seg_len = min(seg_t * 128 + TAU_MAX,
                              128 * NCHUNK - 128 * t0 - 127)
                xd = xdp.tile([128, seg_len], DT_LOW, tag="xd")
                nc.sync.dma_start(
                    xd[:], _ap(xpad8_d.tensor, 128 * t0, [[1, 128], [1, seg_len]])
                )
                for t2 in range(t0 // 2, min((t0 + seg_t) // 2, 312 + 1)):
                    if 2 * t2 > t_eff:
                        break
                    off2 = 256 * t2 - 128 * t0
                    wins = pair_wins.get(t2, ())
                    for a in wins:
                        if a not in cps:
                            cps[a] = psp.tile([WIN, W], F32, tag="c", name=f"c{a}")
                            nc.tensor.matmul(
                                cps[a][:], zl[:], zr[:], start=True, stop=False,
                            )
                        nc.tensor.matmul(
                            cps[a][:],
                            _sap(xb, 128 * t2 + 32 * (a % 2),
                                 [[1, 128], [64, 2], [1, WIN]]),
                            _sap(xd, off2 + 1, [[1, 128], [128, 2], [1, W]]),
                            start=False,
                            stop=(t2 == win_last[a]),
                            perf_mode=DR,
                            skip_group_check=True,
                        )
                    for a in sorted(cps.keys()):
                        if win_last[a] <= t2:
                            b = (WIN * a) // 128
                            r0 = (WIN * a) % 128
                            nc.scalar.copy(csb[b][r0:r0 + WIN, :], cps[a][:])
                            del cps[a]
                            blk_done[b] += 1
                            if blk_done[b] == 4:
                                finish_block(b)

            # ---- gather f0: transpose [128, 8] -> [8, 128], DMA out
            f0t = ps2.tile([N_BLK, 128], F32)
            nc.tensor.transpose(f0t[:], f0all[:, 0:N_BLK], ident[:])
            f0sb = pp.tile([N_BLK, 128], F32)
            nc.scalar.copy(f0sb[:], f0t[:])
            for b in range(N_BLK):
                cnt = 128 if b < N_BLK - 1 else N_OUT - 128 * (N_BLK - 1)
                nc.sync.dma_start(
                    _ap(f0_d, 128 * b, [[1, cnt]]), f0sb[b:b + 1, 0:cnt]
                )

    _split_excess_waits(nc)
    return nc


_NC_CACHE = {}


def _get_nc():
    if "nc" not in _NC_CACHE:
        _NC_CACHE["nc"] = _build_nc()
    return _NC_CACHE["nc"]


def kernel(x: np.ndarray) -> np.ndarray:
    x = np.ascontiguousarray(np.asarray(x), dtype=np.float32)
    assert x.shape == (B, N), x.shape
    nc = _get_nc()
    in_maps = [{"x": x[i]} for i in range(B)]
    res = run_bass_kernel_spmd(nc, in_maps, core_ids=list(range(B)))
    out = np.stack([np.asarray(res.results[i]["f0"]).reshape(N_OUT) for i in range(B)])
    return out.astype(np.float32)



# revision 11
# speedup vs baseline: 1.0668x; 1.0668x over previous
"""YIN pitch Trainium2 kernel, Phase 2: PE band-matmul difference function.

C[f,tau] = sum_n x[n]*x[n+tau]*[80f <= n <= 80f+132] on the tensor engine:
contraction over 128-sample tiles (k = partition = sample), stationary
operand = x-valued band selector slab [128, 32] (<=4 active frame columns,
zero padded; slab positions repeat with period 20 tiles), moving operand =
Hankel slice XD[:, 128t+1 : 128t+134] where XD[p, c] = x[p+c], streamed from
a DRAM bounce buffer in fp8.  PSUM accumulates 32-frame windows (out rows
always [0, 32) - PE requires 32-aligned PSUM base partitions).

Energy terms + CMNDF threshold pick stay on DVE in f32.
"""

import math

import numpy as np

import bass_rust
import concourse.bass as bass
import concourse.mybir as mybir
import concourse.tile as tile
from concourse.bass_utils import run_bass_kernel_spmd
from concourse.tile_rust import add_dep_helper

_WAIT_LIM = 1


def _split_excess_waits(nc):
    uid = 0
    for fn in nc.m.functions:
        for blk in fn.blocks:
            out = []
            changed = False
            for inst in blk.instructions:
                si = inst.sync_info
                waits = list(si.on_wait) if si is not None and si.on_wait else []
                if len(waits) > _WAIT_LIM:
                    changed = True
                    extra = waits[:-_WAIT_LIM]
                    si.on_wait = waits[-_WAIT_LIM:]
                    for j in range(0, len(extra), _WAIT_LIM):
                        nop = bass_rust.InstNoOp(name=f"WSPLIT-{uid}", ins=[], outs=[])
                        uid += 1
                        nop.engine = inst.engine
                        nop.sync_info = bass_rust.SyncInfo(
                            on_wait=extra[j:j + _WAIT_LIM], on_update=[]
                        )
                        out.append(nop)
                out.append(inst)
            if changed:
                blk.instructions = out


def _short_drain_and_barrier(self, tick_clock, wait_clock):
    # Tail with a single all-engine barrier: drain, barrier, sem cleanup.
    # The trailing barrier of the stock TileContext tail only re-syncs
    # engines that have no further work; the runtime joins engines anyway.
    from concourse.vector_clock import ScopedClock
    nc = self.nc
    drain_inst = nc.sync.drain()
    wait_clock.add_sem_waits(
        drain_inst.ins, ScopedClock({None: tick_clock.global_clock})
    )
    nc.all_engine_barrier()
    assert self.sems is not None
    popped = nc._tile_sem_poison_stack.pop()
    assert popped is self._sem_poison
    nc.clear_and_free_semaphores(list(self.sems.allocated().values()))


tile.TileContext._drain_and_barrier = _short_drain_and_barrier


B = 8
N = 80000
SR = 8000
HOP = 80
TAU_MIN = 20
TAU_MAX = 133
W = 133
FRAME_LEN = 266
N_FRAMES = 997
N_OUT = 996          # frames 0..995 are emitted
THRESH = 0.2
EPS = 1e-8
BIG = 1.0e9

N_BLK = 8
FT = 268
G = 4                # max frames per 128-sample tile
NT = 625             # sample tiles
NCHUNK = 640         # xpad chunk width: [128, 640] covers 81920 samples
SEG_T = 128          # tiles per XD segment
SEG_LEN = SEG_T * 128 + TAU_MAX    # 3333
N_SEG = 5
WIN = 32             # frames per PSUM window
PERIOD = 20          # slab-position periodicity in tiles

F32 = mybir.dt.float32
BF16 = mybir.dt.bfloat16
DT_LOW = mybir.dt.float8e4   # PE operand dtype (e4m3); set BF16 to fall back
AluOp = mybir.AluOpType
Axis = mybir.AxisListType


def _ap(t, offset, pairs):
    return bass.AP(t, offset, pairs)


def _sap(tile_ap, offset, pairs):
    """AP on an SBUF tile: partition pair step = row pitch (elements)."""
    pitch = tile_ap[:, 0:1].ap[0][0]
    return bass.AP(tile_ap.tensor, offset, [[pitch, pairs[0][1]]] + pairs[1:])


def _fb(t):
    return math.ceil((128 * t - (W - 1)) / HOP)


def _geometry():
    """Period-5 cover mask + period-20 slab groups."""
    mask5 = np.zeros((128, 5, G), np.float32)
    for r in range(5):
        n0 = 128 * r
        fb = _fb(r)
        for g in range(G):
            f = fb + g
            lo = max(0, HOP * f - n0)
            hi = min(127, HOP * f + (W - 1) - n0)
            if lo <= hi:
                mask5[lo:hi + 1, r, g] = 1.0

    t_eff = max(t for t in range(NT) if _fb(t) <= N_OUT - 1)
    groups = []
    for rho in range(PERIOD):
        fb = _fb(rho)
        byw = {}
        for g in range(G):
            byw.setdefault((fb + g) // WIN, []).append(g)
        for a_off, gs in sorted(byw.items()):
            groups.append(
                dict(rho=rho, a_off=a_off, glo=min(gs), ghi=max(gs),
                     pos=(fb + min(gs)) - WIN * a_off)
            )
    return mask5, groups, t_eff


def _build_nc():
    nc = bass.Bass(trn_type="TRN2")
    x_d = nc.dram_tensor("x", [N], F32, kind="ExternalInput")
    f0_d = nc.dram_tensor("f0", [N_OUT], F32, kind="ExternalOutput")

    mask5, groups, t_eff = _geometry()
    n_groups = len(groups)
    for gi, gr in enumerate(groups):
        gr["nv"] = (t_eff - gr["rho"]) // PERIOD + 1
        gr["gi"] = gi
    by_rho = {}
    for gr in groups:
        by_rho.setdefault(gr["rho"], []).append(gr)

    tau_row = np.arange(1, TAU_MAX + 1, dtype=np.float32)
    tauc_d = nc.inline_tensor(np.broadcast_to(tau_row, (128, W)).copy(), name="tauc")
    taubig_d = nc.inline_tensor(
        (np.broadcast_to(tau_row, (128, W)) + np.float32(BIG)).astype(np.float32),
        name="taubig",
    )
    ident_d = nc.inline_tensor(np.eye(128, dtype=np.float32), name="ident")
    mask_d = nc.inline_tensor(
        mask5.reshape(128, 5 * G).astype(np.float32), name="bmask"
    )
    zl_d = nc.inline_tensor(
        np.zeros((1, WIN), dtype=np.dtype(mybir.dt.np(DT_LOW))), name="zl"
    )
    zr_d = nc.inline_tensor(
        np.zeros((1, W), dtype=np.dtype(mybir.dt.np(DT_LOW))), name="zr"
    )

    # pieces per tile -> windows per pair; win_last in pair units
    def _pieces(t):
        fb = _fb(t)
        byw = {}
        for g in range(G):
            byw.setdefault((fb + g) // WIN, []).append(g)
        return [
            dict(a=a, glo=min(gs), ghi=max(gs), pos=(fb + min(gs)) - WIN * a)
            for a, gs in sorted(byw.items())
        ]

    pair_wins = {}
    win_last = {}
    for t2 in range(313):
        wins = set()
        for t in (2 * t2, 2 * t2 + 1):
            if t > t_eff:
                continue
            for pc in _pieces(t):
                f_lo = max(_fb(t) + pc["glo"], 0)
                f_hi = min(_fb(t) + pc["ghi"], N_OUT - 1)
                if f_lo <= f_hi and pc["a"] >= 0:
                    wins.add(pc["a"])
        if wins:
            pair_wins[t2] = tuple(sorted(wins))
            for a in wins:
                win_last[a] = t2

    with tile.TileContext(nc) as tc:
        with (
            tc.tile_pool(name="persist", bufs=1) as pp,
            tc.tile_pool(name="work", bufs=2) as wp,
            tc.tile_pool(name="xdpool", bufs=3) as xdp,
            tc.tile_pool(name="psum", bufs=6, space="PSUM") as psp,
            tc.tile_pool(name="ps2", bufs=1, space="PSUM") as ps2,
            tc.tile_pool(name="pst", bufs=1, space="PSUM") as pst,
            tc.tile_pool(name="dram", bufs=1, space="DRAM") as dp,
        ):
            # ---- weight-slab zero fill first: overlaps the entire x chain.
            # Split across Pool and Activation so slabs unblock ~2x sooner.
            xb = pp.tile([128, 640 * 64], DT_LOW)
            nc.gpsimd.memset(xb[:, 0:20480].bitcast(F32), 0.0)
            nc.scalar.memzero(xb[:, 20480:40960])

            # ---- x -> chunked SBUF (f32), convert, bounce to DRAM.
            # This chain heads the XD critical path, so it issues before all
            # other DMAs.
            xchunk = pp.tile([128, NCHUNK], F32)
            nc.vector.memset(xchunk[:], 0.0)
            nc.sync.dma_start(
                xchunk[0:125, 0:NCHUNK],
                _ap(x_d, 0, [[NCHUNK, 125], [1, NCHUNK]]),
            )
            xlow = pp.tile([128, NCHUNK], DT_LOW)
            nc.vector.tensor_copy(xlow[:], xchunk[:])
            xpad8_d = dp.tile([128, NCHUNK], DT_LOW)
            nc.sync.dma_start(xpad8_d[:], xlow[:])

            # ---- constants to SBUF
            ident = pp.tile([128, 128], F32)
            nc.sync.dma_start(ident[:], ident_d[:])
            tauc = pp.tile([128, W], F32)
            nc.scalar.dma_start(tauc[:], tauc_d[:])
            taubig = pp.tile([128, W], F32)
            nc.scalar.dma_start(taubig[:], taubig_d[:])
            bmask = pp.tile([128, 5 * G], F32)
            nc.sync.dma_start(bmask[:], mask_d[:])
            zl = pp.tile([1, WIN], DT_LOW)
            nc.scalar.dma_start(zl[:], zl_d[:])
            zr = pp.tile([1, W], DT_LOW)
            nc.scalar.dma_start(zr[:], zr_d[:])
            f0all = pp.tile([128, N_BLK], F32)
            nc.vector.memset(f0all[:], 0.0)

            # ---- xpm[p, m] = x[128 m + p] via 5 on-chip PE transposes of
            # xchunk's 128-col blocks (column m of block j lands at xpm col
            # 5 q + j), replacing the DRAM bf16 bounce + transpose-DMA.
            xpm16 = pp.tile([128, NCHUNK], F32)
            for j in range(5):
                xt = pst.tile([128, 128], F32, tag="xt")
                nc.tensor.transpose(xt[:], xchunk[:, 128 * j:128 * (j + 1)], ident[:])
                nc.scalar.copy(
                    _sap(xpm16, j, [[1, 128], [5, 128]]), xt[:]
                )

            # ---- weight slabs, t-major with window-parity slots:
            # tile t, window a piece -> cols [64 t + 32 (a%2) + pos, +ncols)
            for gr in groups:
                rho, nv = gr["rho"], gr["nv"]
                ncols = gr["ghi"] - gr["glo"] + 1
                for phi in (0, 1):  # v parity (slot alternates with v)
                    nu = (nv - phi + 1) // 2
                    if nu <= 0:
                        continue
                    slot = (gr["a_off"] + phi) % 2
                    base = 64 * (PERIOD * phi + rho) + 32 * slot + gr["pos"]
                    nc.vector.tensor_tensor(
                        out=_sap(xb, base, [[1, 128], [128 * PERIOD, nu], [1, ncols]]),
                        in0=_sap(xpm16, PERIOD * phi + rho,
                                 [[1, 128], [2 * PERIOD, nu], [0, ncols]]),
                        in1=_sap(bmask, (rho % 5) * G + gr["glo"],
                                 [[1, 128], [0, nu], [1, ncols]]),
                        op=AluOp.mult,
                    )
                # clip frames < 0 or > N_OUT-1 (first/last slots only)
                for v in (0, nv - 1):
                    t = PERIOD * v + rho
                    if t > t_eff:
                        continue
                    slot = (gr["a_off"] + v) % 2
                    for g in range(gr["glo"], gr["ghi"] + 1):
                        f = _fb(t) + g
                        if 0 <= f <= N_OUT - 1:
                            continue
                        col = 64 * t + 32 * slot + gr["pos"] + (g - gr["glo"])
                        nc.vector.memset(
                            _sap(xb, col, [[1, 128], [1, 1]]), 0.0
                        )

            csb = {}
            for b in range(N_BLK):
                csb[b] = wp.tile([128, W], F32, tag=f"csb{b}", name=f"csb{b}")
            blk_done = {b: 0 for b in range(N_BLK)}

            def finish_block(b):
                Rb = 128 if b < N_BLK - 1 else N_OUT - 128 * (N_BLK - 1)
                e2 = wp.tile([128, W], F32, tag="e2")
                nc.vector.tensor_sub(
                    e2[:Rb, :], qq[b][:Rb, W:FRAME_LEN], qq[b][:Rb, 0:W]
                )
                d = wp.tile([128, W], F32, tag="d")
                nc.vector.scalar_tensor_tensor(
                    out=d[:Rb, :], in0=csb[b][:Rb, :], scalar=-2.0, in1=e2[:Rb, :],
                    op0=AluOp.mult, op1=AluOp.add,
                )
                nc.vector.tensor_scalar_add(d[:Rb, :], d[:Rb, :], qq[b][:Rb, W - 1:W])
                cum = wp.tile([128, W], F32, tag="cum")
                nc.vector.tensor_tensor_scan(
                    cum[:Rb, :], d[:Rb, :], d[:Rb, :], 0.0, AluOp.add, AluOp.bypass
                )
                lhs = wp.tile([128, W], F32, tag="lhs")
                nc.vector.tensor_mul(lhs[:Rb, :], d[:Rb, :], tauc[:Rb, :])
                rhs = wp.tile([128, W], F32, tag="rhs")
                nc.vector.tensor_scalar(
                    out=rhs[:Rb, :], in0=cum[:Rb, :], scalar1=EPS, scalar2=THRESH,
                    op0=AluOp.max, op1=AluOp.mult,
                )
                cand = wp.tile([128, W], F32, tag="cand")
                nc.vector.tensor_tensor(
                    out=cand[:Rb, :], in0=lhs[:Rb, :], in1=rhs[:Rb, :], op=AluOp.is_lt
                )
                v = wp.tile([128, W], F32, tag="v")
                nc.vector.scalar_tensor_tensor(
                    out=v[:Rb, :], in0=cand[:Rb, :], scalar=-BIG, in1=taubig[:Rb, :],
                    op0=AluOp.mult, op1=AluOp.add,
                )
                tmin = wp.tile([128, 1], F32, tag="tmin")
                nc.vector.tensor_reduce(
                    tmin[:Rb, :], v[:Rb, TAU_MIN - 1:W], axis=Axis.X, op=AluOp.min
                )
                voi = wp.tile([128, 1], F32, tag="voi")
                nc.vector.tensor_scalar(
                    out=voi[:Rb, :], in0=tmin[:Rb, :], scalar1=BIG * 0.5,
                    scalar2=None, op0=AluOp.is_lt,
                )
                rec = wp.tile([128, 1], F32, tag="rec")
                nc.vector.reciprocal(rec[:Rb, :], tmin[:Rb, :])
                f0v = wp.tile([128, 1], F32, tag="f0v")
                nc.vector.tensor_scalar(
                    out=f0v[:Rb, :], in0=rec[:Rb, :], scalar1=float(SR),
                    scalar2=None, op0=AluOp.mult,
                )
                nc.vector.tensor_mul(f0all[:Rb, b:b + 1], f0v[:Rb, :], voi[:Rb, :])

            # ---- band matmuls over XD segments (fp8 DoubleRow pairs)
            cps = {}
            DR = mybir.MatmulPerfMode.DoubleRow
            # progressive segments: small first chunk so matmuls start early
            seg_bounds = [0, 64, 192, 320, 448, 544, 608, 640]
            n_seg = len(seg_bounds) - 1

            def issue_seg(si):
                t0 = seg_bounds[si]
                seg_t = seg_bounds[si + 1] - t0
                seg_len = min(seg_t * 128 + TAU_MAX,
                              128 * NCHUNK - 128 * t0 - 127)
                xd = xdp.tile([128, seg_len], DT_LOW, tag="xd")
                nc.sync.dma_start(
                    xd[:], _ap(xpad8_d.tensor, 128 * t0, [[1, 128], [1, seg_len]])
                )
                return xd

            seg_pending = {0: issue_seg(0), 1: issue_seg(1)}

            # ---- E-path tiles per block (f32): queued after the first two XD
            # segment loads so they don't starve the matmul-feed DMA chain.
            xfr = {}
            qq = {}
            for b in range(N_BLK):
                Rb = 128 if b < N_BLK - 1 else N_OUT - 128 * (N_BLK - 1)
                xfr[b] = wp.tile([128, FT], F32, tag=f"xfr{b}", name=f"xfr{b}")
                nc.scalar.dma_start(
                    xfr[b][:Rb, :],
                    _ap(x_d, HOP * 128 * b, [[HOP, Rb], [1, FT]]),
                )
                sq = wp.tile([128, FRAME_LEN], F32, tag="sq")
                nc.scalar.square(sq[:Rb, :], xfr[b][:Rb, :FRAME_LEN])
                qq[b] = wp.tile([128, FRAME_LEN], F32, tag=f"qq{b}", name=f"qq{b}")
                nc.vector.tensor_tensor_scan(
                    qq[b][:Rb, :], sq[:Rb, :], sq[:Rb, :], 0.0,
                    AluOp.add, AluOp.bypass,
                )

            for si in range(n_seg):
                t0 = seg_bounds[si]
                if t0 > t_eff:
                    break
                seg_t = seg_bounds[si + 1] - t0
                xd = seg_pending.pop(si)
                if si + 2 < n_seg and seg_bounds[si + 2] <= t_eff:
                    seg_pending[si + 2] = issue_seg(si + 2)
                for t2 in range(t0 // 2, min((t0 + seg_t) // 2, 312 + 1)):
                    if 2 * t2 > t_eff:
                        break
                    off2 = 256 * t2 - 128 * t0
                    wins = pair_wins.get(t2, ())
                    for a in wins:
                        if a not in cps:
                            cps[a] = psp.tile([WIN, W], F32, tag="c", name=f"c{a}")
                            nc.tensor.matmul(
                                cps[a][:], zl[:], zr[:], start=True, stop=False,
                            )
                        nc.tensor.matmul(
                            cps[a][:],
                            _sap(xb, 128 * t2 + 32 * (a % 2),
                                 [[1, 128], [64, 2], [1, WIN]]),
                            _sap(xd, off2 + 1, [[1, 128], [128, 2], [1, W]]),
                            start=False,
                            stop=(t2 == win_last[a]),
                            perf_mode=DR,
                            skip_group_check=True,
                        )
                    for a in sorted(cps.keys()):
                        if win_last[a] <= t2:
                            b = (WIN * a) // 128
                            r0 = (WIN * a) % 128
                            nc.scalar.copy(csb[b][r0:r0 + WIN, :], cps[a][:])
                            del cps[a]
                            blk_done[b] += 1
                            if blk_done[b] == 4:
                                finish_block(b)

            # ---- gather f0: transpose [128, 8] -> [8, 128], DMA out
            f0t = ps2.tile([N_BLK, 128], F32)
            nc.tensor.transpose(f0t[:], f0all[:, 0:N_BLK], ident[:])
            f0sb = pp.tile([N_BLK, 128], F32)
            nc.scalar.copy(f0sb[:], f0t[:])
            # one DMA for the full rows, one for the 100-col tail row
            nc.sync.dma_start(
                _ap(f0_d, 0, [[128, N_BLK - 1], [1, 128]]),
                f0sb[0:N_BLK - 1, 0:128],
            )
            nc.sync.dma_start(
                _ap(f0_d, 128 * (N_BLK - 1), [[1, N_OUT - 128 * (N_BLK - 1)]]),
                f0sb[N_BLK - 1:N_BLK, 0:N_OUT - 128 * (N_BLK - 1)],
            )

    _split_excess_waits(nc)
    return nc


_NC_CACHE = {}


def _get_nc():
    if "nc" not in _NC_CACHE:
        _NC_CACHE["nc"] = _build_nc()
    return _NC_CACHE["nc"]


def kernel(x: np.ndarray) -> np.ndarray:
    x = np.ascontiguousarray(np.asarray(x), dtype=np.float32)
    assert x.shape == (B, N), x.shape
    nc = _get_nc()
    in_maps = [{"x": x[i]} for i in range(B)]
    res = run_bass_kernel_spmd(nc, in_maps, core_ids=list(range(B)))
    out = np.stack([np.asarray(res.results[i]["f0"]).reshape(N_OUT) for i in range(B)])
    return out.astype(np.float32)

